# revision 10
# baseline (speedup 1.0000x reference)
"""Fused multi-branch depthwise conv (7x7 + 1x1 + 3x3 + 5x5) for TRN2.

Strategy (v3):
  * The four same-padded depthwise branches merge exactly into ONE 7x7
    depthwise conv: smaller kernels are zero-padded into the center of a
    7x7 kernel; biases sum.
  * Per NeuronCore (8-way batch sharding, 2 images each): for each channel,
    the 7x7 depthwise conv is computed as 7 PSUM-accumulated matmuls on the
    Tensor engine.  Layout: H in partitions, (batch, W) in the free dim.
    lhsT is a banded Toeplitz matrix built from the kernel column K[c,:,dx]
    (host precomputed); rhs is the zero-padded input tile read at free-dim
    offset dx.  Both operands fp16; accumulation is fp32 in PSUM.
  * H=256 = two 122-row chunks + a 12-row tail.  The tail packs 7 channels
    per block-diagonal matmul; its rhs is ONE host-packed DRAM tensor
    loaded in a single DMA.  Tail matmul groups are interleaved between
    main groups so the PE never idles.
  * Short dummy-LDWEIGHTS bursts after each 28-matmul block drop PE-array
    duty just enough to avoid the chip's sustained-load downclock
    (measured: at 100% PE duty all engine clocks drop x5/6 -> matmuls
    259 ns instead of 216 ns).
  * Output is fp16 in an SBUF-layout-contiguous DRAM tensor (one 0.5 MB
    line-rate store per group-chunk); the host transposes to [B,C,H,W]
    fp32.  Halves store traffic vs fp32 - needed because at full speed
    total HBM traffic would exceed the 358 GB/s/core HBM limit.
  * PSUM->SBUF eviction (bias add + fp32->fp16 cast) alternates between
    the Vector and Scalar engines.
  * Queues: rhs loads on Sync (HWDGE); band loads (prefetched 2 groups
    ahead) + main stores on Scalar (HWDGE); tail loads on GpSimd (SWDGE).
    Group 0's band/rhs loads are split per-channel so the first matmul
    only waits for ~360 KB, not the whole prefetch burst.
"""

import os
import numpy as np
from contextlib import ExitStack

import concourse.bass as bass
import concourse.tile as tile
from concourse import bacc, mybir
from concourse.bass_utils import run_bass_kernel_spmd

N_CORES = 8
B, C, H, W = 16, 64, 256, 256
PB = B // N_CORES            # images per core
HP, WP = H + 6, W + 8        # padded input: 3+3 rows, 3 left + 5 right cols
CG = 4                       # channels per DMA/compute group
NG = C // CG
MAIN_CHUNKS = [0, 122]       # output row chunk starts (122 rows each)
TAIL_H0 = 244                # tail: out rows [244,256) (12), in hp rows [244,262) (18)
TAIL_OR, TAIL_IR = 12, 18
TAIL_GROUPS = [list(range(g, min(g + 7, C))) for g in range(0, C, 7)]
NTG = len(TAIL_GROUPS)       # 10

LAST_RESULTS = None  # BassKernelResults of the most recent run (for test.py)

_prog_cache = {}


def _build_program():
    if "nc" in _prog_cache:
        return _prog_cache["nc"]
    f32 = mybir.dt.float32
    f16 = mybir.dt.float16
    GAP_N = int(os.environ.get("PE_GAP_LDWS", "4"))
    GAP_EVERY = int(os.environ.get("PE_GAP_EVERY", "1"))

    nc = bacc.Bacc("TRN2", target_bir_lowering=False, debug=False)
    xp = nc.dram_tensor("xp", [NG, HP, CG, PB, WP], f16, kind="ExternalInput").ap()
    xt = nc.dram_tensor("xt", [126, NTG, PB, WP], f16, kind="ExternalInput").ap()
    bands = nc.dram_tensor("bands", [NG, 128, CG, 7, 128], f16, kind="ExternalInput").ap()
    tbands = nc.dram_tensor("tbands", [NTG, 126, 7, 84], f16, kind="ExternalInput").ap()
    bb = nc.dram_tensor("bb", [128, C], f32, kind="ExternalInput").ap()
    tbb = nc.dram_tensor("tbb", [128, NTG], f32, kind="ExternalInput").ap()
    # main output, SBUF-tile order: [g, chunk, h, c_in_group, b, w] fp16
    ym = nc.dram_tensor("ym", [NG, 2, 122, CG, PB, W], f16, kind="ExternalOutput").ap()
    # tail output, collect-tile order: [row=(j*12+r), tg, b, w] fp16
    yt = nc.dram_tensor("yt", [84, NTG, PB, W], f16, kind="ExternalOutput").ap()

    # tail group tg runs after main group TAIL_AFTER[tg]'s chunks
    TAIL_AFTER = {tg + 2: tg for tg in range(NTG)}  # groups 2..11

    with tile.TileContext(nc) as tc, ExitStack() as ctx:
        bias_pool = ctx.enter_context(tc.tile_pool(name="bias", bufs=1))
        tailin_pool = ctx.enter_context(tc.tile_pool(name="tailin", bufs=1))
        band_pool = ctx.enter_context(tc.tile_pool(name="band", bufs=4))
        rhs_pool = ctx.enter_context(tc.tile_pool(name="rhs", bufs=4))
        out_pool = ctx.enter_context(tc.tile_pool(name="out", bufs=4))
        psum_pool = ctx.enter_context(tc.tile_pool(name="psum", bufs=8, space="PSUM"))

        bb_t = bias_pool.tile([128, C], f32)
        nc.sync.dma_start(bb_t[:], bb[:])
        tbb_t = bias_pool.tile([128, NTG], f32)
        nc.sync.dma_start(tbb_t[:], tbb[:])

        # tail inputs + tail bands + tail collect tile (SWDGE queue)
        xt_t = tailin_pool.tile([126, NTG, PB, WP], f16)
        nc.gpsimd.dma_start(xt_t[:], xt[:])
        tband_t = tailin_pool.tile([126, NTG, 7, 84], f16)
        nc.gpsimd.dma_start(tband_t[:], tbands.rearrange("g p d o -> p g d o"))
        tcol_t = tailin_pool.tile([84, NTG, PB, W], f16)

        def tail_group(tg):
            chans = TAIL_GROUPS[tg]
            pr, po = len(chans) * TAIL_IR, len(chans) * TAIL_OR
            pt = psum_pool.tile([128, PB, W], f32)
            for dx in range(7):
                nc.tensor.matmul(
                    pt[:po],
                    tband_t[:pr, tg, dx, :po],
                    xt_t[:pr, tg, :, dx : dx + W],
                    start=(dx == 0),
                    stop=(dx == 6),
                )
            nc.vector.tensor_scalar_add(
                tcol_t[:po, tg], pt[:po], tbb_t[:po, tg : tg + 1]
            )

        # Band tiles, prefetched 2 groups ahead on the Scalar HWDGE queue.
        # Group 0's is split per-channel so the first matmul group's weights
        # arrive ASAP.
        band_tiles = {}

        def load_band(g):
            t = band_pool.tile([128, CG, 7, 128], f16, tag="band")
            if g == 0:
                for cj in range(CG):
                    nc.scalar.dma_start(t[:, cj], bands[g, :, cj])
            else:
                nc.scalar.dma_start(t[:], bands[g])
            band_tiles[g] = t

        for g in range(min(3, NG)):
            load_band(g)

        # Main body: two 122-row output chunks per 4-channel group.
        for g in range(NG):
            if g + 3 <= NG - 1:
                load_band(g + 3)
            band_t = band_tiles.pop(g)
            for ci, h0 in enumerate(MAIN_CHUNKS):
                rt = rhs_pool.tile([128, CG, PB, WP], f16, tag="rhs")
                if g == 0 and ci == 0:
                    for cj in range(CG):
                        nc.sync.dma_start(rt[:, cj], xp[g, h0 : h0 + 128, cj])
                else:
                    nc.sync.dma_start(rt[:], xp[g, h0 : h0 + 128])
                ot = out_pool.tile([128, CG, PB, W], f16, tag="out")
                for cj in range(CG):
                    pt = psum_pool.tile([128, PB, W], f32)
                    for dx in range(7):
                        # full 128-col lhsT so fast-weight-load fires; psum
                        # rows >= 122 hold partial sums and are never read
                        nc.tensor.matmul(
                            pt[:],
                            band_t[:, cj, dx, :],
                            rt[:, cj, :, dx : dx + W],
                            start=(dx == 0),
                            stop=(dx == 6),
                        )
                    c = g * CG + cj
                    if cj % 2 == 0 or os.environ.get("EVICT_MODE") == "vector":
                        nc.vector.tensor_scalar_add(
                            ot[:122, cj], pt[:122], bb_t[:122, c : c + 1]
                        )
                    else:
                        nc.scalar.add(ot[:122, cj], pt[:122], bb_t[:122, c : c + 1])
                nc.scalar.dma_start(ym[g, ci], ot[:122])
                # PE-array idle gap (dummy weight loads): sheds enough power
                # to keep the core out of the sustained-load downclock
                # (100% PE duty -> clocks drop 2.4 -> 2.0 GHz chip-wide).
                if (2 * g + ci) % GAP_EVERY == 0:
                    for _ in range(GAP_N):
                        nc.tensor.ldweights(band_t[:, 0, 0, :])
            if g in TAIL_AFTER:
                tail_group(TAIL_AFTER[g])
                if TAIL_AFTER[g] == NTG - 1:
                    nc.sync.dma_start(yt[:], tcol_t[:])

    nc.compile()
    _prog_cache["nc"] = nc
    return nc


def kernel(x, w7, b7, w1, b1, w3, b3, w5, b5):
    global LAST_RESULTS
    x = np.asarray(x, dtype=np.float32)
    # Merge the four branches into one 7x7 depthwise kernel + one bias.
    K = np.asarray(w7, dtype=np.float32)[:, 0].copy()          # [C,7,7]
    K[:, 3:4, 3:4] += np.asarray(w1, dtype=np.float32)[:, 0]
    K[:, 2:5, 2:5] += np.asarray(w3, dtype=np.float32)[:, 0]
    K[:, 1:6, 1:6] += np.asarray(w5, dtype=np.float32)[:, 0]
    b_m = (
        np.asarray(b7, dtype=np.float32)
        + np.asarray(b1, dtype=np.float32)
        + np.asarray(b3, dtype=np.float32)
        + np.asarray(b5, dtype=np.float32)
    )

    # Zero-padded fp16 input, packed per core as [NG, HP, CG, PB, WP]
    xp = np.zeros((B, C, HP, WP), dtype=np.float16)
    xp[:, :, 3 : 3 + H, 3 : 3 + W] = x.astype(np.float16)
    xp_packed = [
        np.ascontiguousarray(
            xp[i * PB : (i + 1) * PB]
            .reshape(PB, NG, CG, HP, WP)
            .transpose(1, 3, 2, 0, 4)
        )
        for i in range(N_CORES)
    ]
    # Tail rhs, packed per core as [126, NTG, PB, WP]
    xt_packed = []
    for i in range(N_CORES):
        xt_i = np.zeros((126, NTG, PB, WP), dtype=np.float16)
        xpc = xp[i * PB : (i + 1) * PB]                        # [PB, C, HP, WP]
        for tg, chans in enumerate(TAIL_GROUPS):
            for j, c in enumerate(chans):
                xt_i[j * TAIL_IR : (j + 1) * TAIL_IR, tg] = xpc[
                    :, c, TAIL_H0 : TAIL_H0 + TAIL_IR
                ].transpose(1, 0, 2)
        xt_packed.append(xt_i)

    # Banded Toeplitz weights: band[i, dx, o] = K[i-o, dx] for i-o in [0,7)
    def band_block(Kc, n_in, n_out):
        ii = np.arange(n_in)[:, None]
        oo = np.arange(n_out)[None, :]
        d = ii - oo
        mask = (d >= 0) & (d < 7)
        dcl = np.clip(d, 0, 6)
        blk = Kc[dcl, :] * mask[:, :, None]    # [n_in, n_out, 7]
        return blk.transpose(0, 2, 1)          # [n_in, 7, n_out]

    bands = np.zeros((NG, 128, CG, 7, 128), dtype=np.float16)
    for c in range(C):
        bands[c // CG, :, c % CG] = band_block(K[c], 128, 128).astype(np.float16)

    tbands = np.zeros((NTG, 126, 7, 84), dtype=np.float16)
    tbb = np.zeros((128, NTG), dtype=np.float32)
    for g, chans in enumerate(TAIL_GROUPS):
        for j, c in enumerate(chans):
            tbands[
                g, j * TAIL_IR : (j + 1) * TAIL_IR, :, j * TAIL_OR : (j + 1) * TAIL_OR
            ] = band_block(K[c], TAIL_IR, TAIL_OR).astype(np.float16)
            tbb[j * TAIL_OR : (j + 1) * TAIL_OR, g] = b_m[c]

    bb = np.ascontiguousarray(np.broadcast_to(b_m, (128, C)), dtype=np.float32)

    nc = _build_program()
    in_maps = [
        {
            "xp": xp_packed[i],
            "xt": xt_packed[i],
            "bands": bands,
            "tbands": tbands,
            "bb": bb,
            "tbb": tbb,
        }
        for i in range(N_CORES)
    ]
    LAST_RESULTS = run_bass_kernel_spmd(nc, in_maps, list(range(N_CORES)))

    out = np.empty((B, C, H, W), dtype=np.float32)
    for i in range(N_CORES):
        res = LAST_RESULTS.results[i]
        ym_r = res["ym"]                       # [NG, 2, 122, CG, PB, W] f16
        yt_r = res["yt"]                       # [84, NTG, PB, W] f16
        oc = out[i * PB : (i + 1) * PB]
        # [NG,2,122,CG,PB,W] -> [PB, NG, CG, 2, 122, W] -> [PB, C, 244, W]
        oc[:, :, :244] = (
            ym_r.transpose(4, 0, 3, 1, 2, 5).reshape(PB, C, 244, W)
        )
        for tg, chans in enumerate(TAIL_GROUPS):
            for j, c in enumerate(chans):
                oc[:, c, TAIL_H0:] = yt_r[
                    j * TAIL_OR : (j + 1) * TAIL_OR, tg
                ].transpose(1, 0, 2)
    return out


# revision 12
# speedup vs baseline: 1.6390x; 1.6390x over previous
"""Fused multi-branch depthwise conv (7x7 + 1x1 + 3x3 + 5x5) for TRN2.

Strategy (v3):
  * The four same-padded depthwise branches merge exactly into ONE 7x7
    depthwise conv: smaller kernels are zero-padded into the center of a
    7x7 kernel; biases sum.
  * Per NeuronCore (8-way batch sharding, 2 images each): for each channel,
    the 7x7 depthwise conv is computed as 7 PSUM-accumulated matmuls on the
    Tensor engine.  Layout: H in partitions, (batch, W) in the free dim.
    lhsT is a banded Toeplitz matrix built from the kernel column K[c,:,dx]
    (host precomputed); rhs is the zero-padded input tile read at free-dim
    offset dx.  Both operands fp16; accumulation is fp32 in PSUM.
  * H=256 = two 122-row chunks + a 12-row tail.  The tail packs 7 channels
    per block-diagonal matmul; its rhs is ONE host-packed DRAM tensor
    loaded in a single DMA.  Tail matmul groups are interleaved between
    main groups so the PE never idles.
  * Short dummy-LDWEIGHTS bursts after each 28-matmul block drop PE-array
    duty just enough to avoid the chip's sustained-load downclock
    (measured: at 100% PE duty all engine clocks drop x5/6 -> matmuls
    259 ns instead of 216 ns).
  * Output is fp16 in an SBUF-layout-contiguous DRAM tensor (one 0.5 MB
    line-rate store per group-chunk); the host transposes to [B,C,H,W]
    fp32.  Halves store traffic vs fp32 - needed because at full speed
    total HBM traffic would exceed the 358 GB/s/core HBM limit.
  * PSUM->SBUF eviction (bias add + fp32->fp16 cast) alternates between
    the Vector and Scalar engines.
  * Queues: rhs loads on Sync (HWDGE); band loads (prefetched 2 groups
    ahead) + main stores on Scalar (HWDGE); tail loads on GpSimd (SWDGE).
    Group 0's band/rhs loads are split per-channel so the first matmul
    only waits for ~360 KB, not the whole prefetch burst.
"""

import os
import numpy as np
from contextlib import ExitStack

import concourse.bass as bass
import concourse.tile as tile
from concourse import bacc, mybir
from concourse.bass_utils import run_bass_kernel_spmd

N_CORES = 8
B, C, H, W = 16, 64, 256, 256
PB = B // N_CORES            # images per core
HP, WP = H + 6, W + 8        # padded input: 3+3 rows, 3 left + 5 right cols
CG = 4                       # channels per DMA/compute group
NG = C // CG
MAIN_CHUNKS = [0, 122]       # output row chunk starts (122 rows each)
TAIL_H0 = 244                # tail: out rows [244,256) (12), in hp rows [244,262) (18)
TAIL_OR, TAIL_IR = 12, 18
TAIL_GROUPS = [list(range(g, min(g + 7, C))) for g in range(0, C, 7)]
NTG = len(TAIL_GROUPS)       # 10

LAST_RESULTS = None  # BassKernelResults of the most recent run (for test.py)

_prog_cache = {}


def _build_program():
    if "nc" in _prog_cache:
        return _prog_cache["nc"]
    f32 = mybir.dt.float32
    f16 = mybir.dt.float16
    GAP_N = int(os.environ.get("PE_GAP_LDWS", "4"))
    GAP_EVERY = int(os.environ.get("PE_GAP_EVERY", "1"))

    nc = bacc.Bacc("TRN2", target_bir_lowering=False, debug=False)
    xp = nc.dram_tensor("xp", [NG, HP, CG, PB, WP], f16, kind="ExternalInput").ap()
    xt = nc.dram_tensor("xt", [126, NTG, PB, WP], f16, kind="ExternalInput").ap()
    bands = nc.dram_tensor("bands", [NG, 128, CG, 7, 128], f16, kind="ExternalInput").ap()
    tbands = nc.dram_tensor("tbands", [NTG, 126, 7, 84], f16, kind="ExternalInput").ap()
    bb = nc.dram_tensor("bb", [128, C], f32, kind="ExternalInput").ap()
    tbb = nc.dram_tensor("tbb", [128, NTG], f32, kind="ExternalInput").ap()
    # main output, SBUF-tile order: [g, chunk, h, c_in_group, b, w] fp16
    ym = nc.dram_tensor("ym", [NG, 2, 122, CG, PB, W], f16, kind="ExternalOutput").ap()
    # tail output, collect-tile order: [row=(j*12+r), tg, b, w] fp16
    yt = nc.dram_tensor("yt", [84, NTG, PB, W], f16, kind="ExternalOutput").ap()

    # tail group tg runs after main group TAIL_AFTER[tg]'s chunks
    TAIL_AFTER = {tg + 2: tg for tg in range(NTG)}  # groups 2..11

    with tile.TileContext(nc) as tc, ExitStack() as ctx:
        bias_pool = ctx.enter_context(tc.tile_pool(name="bias", bufs=1))
        tailin_pool = ctx.enter_context(tc.tile_pool(name="tailin", bufs=1))
        band_pool = ctx.enter_context(tc.tile_pool(name="band", bufs=4))
        rhs_pool = ctx.enter_context(tc.tile_pool(name="rhs", bufs=4))
        out_pool = ctx.enter_context(tc.tile_pool(name="out", bufs=4))
        psum_pool = ctx.enter_context(tc.tile_pool(name="psum", bufs=8, space="PSUM"))

        bb_t = bias_pool.tile([128, C], f32)
        nc.sync.dma_start(bb_t[:], bb[:])
        tbb_t = bias_pool.tile([128, NTG], f32)
        nc.sync.dma_start(tbb_t[:], tbb[:])

        # tail inputs + tail bands + tail collect tile (SWDGE queue)
        xt_t = tailin_pool.tile([126, NTG, PB, WP], f16)
        nc.gpsimd.dma_start(xt_t[:], xt[:])
        tband_t = tailin_pool.tile([126, NTG, 7, 84], f16)
        nc.gpsimd.dma_start(tband_t[:], tbands.rearrange("g p d o -> p g d o"))
        tcol_t = tailin_pool.tile([84, NTG, PB, W], f16)

        def tail_group(tg):
            chans = TAIL_GROUPS[tg]
            pr, po = len(chans) * TAIL_IR, len(chans) * TAIL_OR
            pt = psum_pool.tile([128, PB, W], f32)
            for dx in range(7):
                nc.tensor.matmul(
                    pt[:po],
                    tband_t[:pr, tg, dx, :po],
                    xt_t[:pr, tg, :, dx : dx + W],
                    start=(dx == 0),
                    stop=(dx == 6),
                )
            nc.vector.tensor_scalar_add(
                tcol_t[:po, tg], pt[:po], tbb_t[:po, tg : tg + 1]
            )

        # Band tiles, prefetched 2 groups ahead on the Scalar HWDGE queue.
        # Group 0's is split per-channel so the first matmul group's weights
        # arrive ASAP.
        band_tiles = {}

        def load_band(g):
            t = band_pool.tile([128, CG, 7, 128], f16, tag="band")
            if g == 0:
                for cj in range(CG):
                    nc.scalar.dma_start(t[:, cj], bands[g, :, cj])
            else:
                nc.scalar.dma_start(t[:], bands[g])
            band_tiles[g] = t

        for g in range(min(3, NG)):
            load_band(g)

        # Main body: two 122-row output chunks per 4-channel group.
        for g in range(NG):
            if g + 3 <= NG - 1:
                load_band(g + 3)
            band_t = band_tiles.pop(g)
            for ci, h0 in enumerate(MAIN_CHUNKS):
                rt = rhs_pool.tile([128, CG, PB, WP], f16, tag="rhs")
                if g == 0 and ci == 0:
                    for cj in range(CG):
                        nc.sync.dma_start(rt[:, cj], xp[g, h0 : h0 + 128, cj])
                else:
                    nc.sync.dma_start(rt[:], xp[g, h0 : h0 + 128])
                ot = out_pool.tile([128, CG, PB, W], f16, tag="out")
                for cj in range(CG):
                    pt = psum_pool.tile([128, PB, W], f32)
                    for dx in range(7):
                        # full 128-col lhsT so fast-weight-load fires; psum
                        # rows >= 122 hold partial sums and are never read
                        nc.tensor.matmul(
                            pt[:],
                            band_t[:, cj, dx, :],
                            rt[:, cj, :, dx : dx + W],
                            start=(dx == 0),
                            stop=(dx == 6),
                        )
                    c = g * CG + cj
                    if cj % 2 == 0 or os.environ.get("EVICT_MODE") == "vector":
                        nc.vector.tensor_scalar_add(
                            ot[:122, cj], pt[:122], bb_t[:122, c : c + 1]
                        )
                    else:
                        nc.scalar.add(ot[:122, cj], pt[:122], bb_t[:122, c : c + 1])
                nc.gpsimd.dma_start(ym[g, ci], ot[:122])
                # PE-array idle gap (dummy weight loads): sheds enough power
                # to keep the core out of the sustained-load downclock
                # (100% PE duty -> clocks drop 2.4 -> 2.0 GHz chip-wide).
                if (2 * g + ci) % GAP_EVERY == 0:
                    for _ in range(GAP_N):
                        nc.tensor.ldweights(band_t[:, 0, 0, :])
            if g in TAIL_AFTER:
                tail_group(TAIL_AFTER[g])
                if TAIL_AFTER[g] == NTG - 1:
                    nc.gpsimd.dma_start(yt[:], tcol_t[:])

    nc.compile()
    _prog_cache["nc"] = nc
    return nc


def kernel(x, w7, b7, w1, b1, w3, b3, w5, b5):
    global LAST_RESULTS
    x = np.asarray(x, dtype=np.float32)
    # Merge the four branches into one 7x7 depthwise kernel + one bias.
    K = np.asarray(w7, dtype=np.float32)[:, 0].copy()          # [C,7,7]
    K[:, 3:4, 3:4] += np.asarray(w1, dtype=np.float32)[:, 0]
    K[:, 2:5, 2:5] += np.asarray(w3, dtype=np.float32)[:, 0]
    K[:, 1:6, 1:6] += np.asarray(w5, dtype=np.float32)[:, 0]
    b_m = (
        np.asarray(b7, dtype=np.float32)
        + np.asarray(b1, dtype=np.float32)
        + np.asarray(b3, dtype=np.float32)
        + np.asarray(b5, dtype=np.float32)
    )

    # Zero-padded fp16 input, packed per core as [NG, HP, CG, PB, WP]
    xp = np.zeros((B, C, HP, WP), dtype=np.float16)
    xp[:, :, 3 : 3 + H, 3 : 3 + W] = x.astype(np.float16)
    xp_packed = [
        np.ascontiguousarray(
            xp[i * PB : (i + 1) * PB]
            .reshape(PB, NG, CG, HP, WP)
            .transpose(1, 3, 2, 0, 4)
        )
        for i in range(N_CORES)
    ]
    # Tail rhs, packed per core as [126, NTG, PB, WP]
    xt_packed = []
    for i in range(N_CORES):
        xt_i = np.zeros((126, NTG, PB, WP), dtype=np.float16)
        xpc = xp[i * PB : (i + 1) * PB]                        # [PB, C, HP, WP]
        for tg, chans in enumerate(TAIL_GROUPS):
            for j, c in enumerate(chans):
                xt_i[j * TAIL_IR : (j + 1) * TAIL_IR, tg] = xpc[
                    :, c, TAIL_H0 : TAIL_H0 + TAIL_IR
                ].transpose(1, 0, 2)
        xt_packed.append(xt_i)

    # Banded Toeplitz weights: band[i, dx, o] = K[i-o, dx] for i-o in [0,7)
    def band_block(Kc, n_in, n_out):
        ii = np.arange(n_in)[:, None]
        oo = np.arange(n_out)[None, :]
        d = ii - oo
        mask = (d >= 0) & (d < 7)
        dcl = np.clip(d, 0, 6)
        blk = Kc[dcl, :] * mask[:, :, None]    # [n_in, n_out, 7]
        return blk.transpose(0, 2, 1)          # [n_in, 7, n_out]

    bands = np.zeros((NG, 128, CG, 7, 128), dtype=np.float16)
    for c in range(C):
        bands[c // CG, :, c % CG] = band_block(K[c], 128, 128).astype(np.float16)

    tbands = np.zeros((NTG, 126, 7, 84), dtype=np.float16)
    tbb = np.zeros((128, NTG), dtype=np.float32)
    for g, chans in enumerate(TAIL_GROUPS):
        for j, c in enumerate(chans):
            tbands[
                g, j * TAIL_IR : (j + 1) * TAIL_IR, :, j * TAIL_OR : (j + 1) * TAIL_OR
            ] = band_block(K[c], TAIL_IR, TAIL_OR).astype(np.float16)
            tbb[j * TAIL_OR : (j + 1) * TAIL_OR, g] = b_m[c]

    bb = np.ascontiguousarray(np.broadcast_to(b_m, (128, C)), dtype=np.float32)

    nc = _build_program()
    in_maps = [
        {
            "xp": xp_packed[i],
            "xt": xt_packed[i],
            "bands": bands,
            "tbands": tbands,
            "bb": bb,
            "tbb": tbb,
        }
        for i in range(N_CORES)
    ]
    LAST_RESULTS = run_bass_kernel_spmd(nc, in_maps, list(range(N_CORES)))

    out = np.empty((B, C, H, W), dtype=np.float32)
    for i in range(N_CORES):
        res = LAST_RESULTS.results[i]
        ym_r = res["ym"]                       # [NG, 2, 122, CG, PB, W] f16
        yt_r = res["yt"]                       # [84, NTG, PB, W] f16
        oc = out[i * PB : (i + 1) * PB]
        # [NG,2,122,CG,PB,W] -> [PB, NG, CG, 2, 122, W] -> [PB, C, 244, W]
        oc[:, :, :244] = (
            ym_r.transpose(4, 0, 3, 1, 2, 5).reshape(PB, C, 244, W)
        )
        for tg, chans in enumerate(TAIL_GROUPS):
            for j, c in enumerate(chans):
                oc[:, c, TAIL_H0:] = yt_r[
                    j * TAIL_OR : (j + 1) * TAIL_OR, tg
                ].transpose(1, 0, 2)
    return out
